# revision 3
# baseline (speedup 1.0000x reference)
"""GATv2 edge-score kernel for 8 TRN2 NeuronCores (two-hop one-hot, no gather).

Math: the reference's layer loop is idempotent (h never changes) and eh is
unused, so the output is one pass:
    h   = node_feat @ W_node + b_node                       [N, C]
    e_j = leaky_relu(cat(h[src_j], h[dst_j]) @ W_a1 + b_a1) @ W_a2 + b_a2

Factored into per-node tables (A = h@W_a1[:C] + b_a1, B = h@W_a1[C:]) with
|w2| folded in (leaky_relu is positively homogeneous):
    e_j = sum_{c in pos} lrelu(u_jc) - sum_{c in neg} lrelu(u_jc) + b_a2
    u_j = A[src_j] + B[dst_j]           (channels permuted pos-first)

v2 (this file): BOTH gathers are one-hot matmuls.  Nodes are padded to 80
windows of 128.  Edges are grouped by (src-window ws, dst-window wd) blocks;
a 128-edge tile's u is two PE matmuls accumulating in PSUM:
    psum[slot, c] = ohA.T @ A_window(ws)  +  ohB.T @ B_window(wd)
One-hots are fp8e4 (exact for 0/1, halves DMA); tables are bf16.
This removes the dma_gather (272us of GpSimd descgen in v1) and the
psum+gather add (199us of DVE) entirely.

SPMD trick: edge -> core by diagonal band c = ((wd - ws) mod 80) // 10, so
every core's tile list k has the SAME table indices a_k = ws and
b_k = (ws + j) mod 80.  Core c's B-table is built from nfT2 = node features
with window blocks rotated by 10c, so SBUF index b holds physical window
(b + 10c) mod 80.  The rotation lives in input data; the program is shared.
"""

import os
import numpy as np
import ml_dtypes

BF16 = ml_dtypes.bfloat16
FP8 = ml_dtypes.float8_e4m3

# ---- problem constants (hardcoded; grader supplies exactly this shape) ----
N_NODES = 10000
N_FEAT = 118
CH = 128
N_EDGES = 640000
N_CORES = 8
NW = 80                      # node windows of 128
NODE_PAD = NW * 128          # 10240
BAND = NW // N_CORES         # 10 dst-window diagonals per core
GT = 4                       # tiles per PSUM group (1 bank)
CHUNK = 32                   # tiles per one-hot DMA chunk


def plan_shards(src, dst):
    """Diagonal-band core assignment with a shared tile template.

    Returns (a_k, b_k, T, slot_edge) where tile k on every core uses
    A-window a_k and B-table index b_k, and slot_edge[c] maps slot ->
    global edge id (-1 pad).
    """
    ws = (src // 128).astype(np.int64)
    wd = (dst // 128).astype(np.int64)
    d = (wd - ws) % NW
    core = d // BAND
    j = d % BAND
    m = ws * BAND + j                      # template slot, 0..799

    cnt = np.bincount(m * N_CORES + core,
                      minlength=NW * BAND * N_CORES).reshape(-1, N_CORES)
    mx = cnt.max(axis=1)
    Q = np.where(mx > 0, -(-mx // 128), 0).astype(np.int64)   # tiles per slot
    K = np.concatenate([[0], np.cumsum(Q)])                   # tile base
    T_raw = int(K[-1])
    T = -(-T_raw // GT) * GT

    a_k = np.zeros(T, np.int64)
    b_k = np.zeros(T, np.int64)
    mm = np.nonzero(Q > 0)[0]
    tile_m = np.repeat(mm, Q[mm])
    a_k[:T_raw] = tile_m // BAND
    b_k[:T_raw] = (tile_m // BAND + tile_m % BAND) % NW

    # per-core slot assignment: sort edges by (core, m), rank within group
    order = np.lexsort((m, core))
    mo, co = m[order], core[order]
    gid = co * (NW * BAND) + mo
    start = np.zeros(len(gid), bool)
    start[0] = True
    start[1:] = gid[1:] != gid[:-1]
    gstart = np.nonzero(start)[0]
    rank = np.arange(len(gid)) - np.repeat(gstart, np.diff(
        np.concatenate([gstart, [len(gid)]])))
    slot = K[mo] * 128 + rank

    slot_edge = np.full((N_CORES, T * 128), -1, np.int64)
    slot_edge[co, slot] = order
    return a_k, b_k, T, slot_edge


def build_program(cfg, p_pos, b_a2, a_k, b_k, T):
    """One SPMD Bass program shared by all cores."""
    import concourse.mybir as mybir
    import concourse.tile as tile
    from concourse import bacc

    f32 = mybir.dt.float32
    bf16 = mybir.dt.bfloat16
    fp8 = mybir.dt.float8e4
    AF = mybir.ActivationFunctionType

    nf = cfg["n_feat"]
    ch = cfg["ch"]
    npad = cfg["n_node_pad"]
    nw = npad // 128
    kdim = nf + 1
    assert T % GT == 0

    nc = bacc.Bacc("TRN2", target_bir_lowering=False)
    nfT = nc.declare_dram_parameter("nfT", [kdim, npad], bf16, isOutput=False)
    nfT2 = nc.declare_dram_parameter("nfT2", [kdim, npad], bf16,
                                     isOutput=False)
    Wn = nc.declare_dram_parameter("Wn", [kdim, ch], bf16, isOutput=False)
    Wa1s = nc.declare_dram_parameter("Wa1s", [ch, ch], bf16, isOutput=False)
    Wa1d = nc.declare_dram_parameter("Wa1d", [ch, ch], bf16, isOutput=False)
    biasA = nc.declare_dram_parameter("biasA", [128, 16 * ch], f32,
                                      isOutput=False)
    oh = nc.declare_dram_parameter("ohAB", [128, T * 2 * 128], fp8,
                                   isOutput=False)
    outp = nc.declare_dram_parameter("out", [128, T], f32, isOutput=True)

    with tile.TileContext(nc) as tc:
        with tc.tile_pool(name="persist", bufs=1) as pers:
            tabA_sb = pers.tile([128, nw, ch], bf16)
            tabB_sb = pers.tile([128, nw, ch], bf16)
            out_sb = pers.tile([128, T], f32)
            rp = pers.tile([128, T], f32)
            rn = pers.tile([128, T], f32)
            biasA_sb = pers.tile([128, 16 * ch], f32)
            nc.sync.dma_start(biasA_sb[:], biasA[:])

            with tc.tile_pool(name="pre", bufs=1) as pre, \
                 tc.tile_pool(name="psum_pre", bufs=2, space="PSUM") as psum:
                nfT_sb = pre.tile([kdim, npad], bf16)
                nc.sync.dma_start(nfT_sb[:], nfT[:])
                nfT2_sb = pre.tile([kdim, npad], bf16)
                nc.sync.dma_start(nfT2_sb[:], nfT2[:])
                Wn_sb = pre.tile([kdim, ch], bf16)
                nc.sync.dma_start(Wn_sb[:], Wn[:])
                Wa1s_sb = pre.tile([ch, ch], bf16)
                nc.sync.dma_start(Wa1s_sb[:], Wa1s[:])
                Wa1d_sb = pre.tile([ch, ch], bf16)
                nc.sync.dma_start(Wa1d_sb[:], Wa1d[:])
                hT_sb = pre.tile([ch, npad], bf16)
                hT2_sb = pre.tile([ch, npad], bf16)

                # hT[c, n] = (node_feat @ W_node + b_node).T via ones-row
                HCH = 512
                for hsrc, hdst in ((nfT_sb, hT_sb), (nfT2_sb, hT2_sb)):
                    for c0 in range(0, npad, HCH):
                        ph = psum.tile([ch, HCH], f32, tag="ph")
                        nc.tensor.matmul(ph[:], Wn_sb[:],
                                         hsrc[:, c0:c0 + HCH],
                                         start=True, stop=True)
                        nc.vector.tensor_copy(hdst[:, c0:c0 + HCH], ph[:])

                # A-table (bias folded) and B-table, 8 windows per group
                for g0 in range(0, nw, 8):
                    pa = psum.tile([128, 8 * ch], f32, tag="pt")
                    for jj in range(8):
                        w = g0 + jj
                        nc.tensor.matmul(pa[:, jj * ch:(jj + 1) * ch],
                                         hT_sb[:, w * 128:(w + 1) * 128],
                                         Wa1s_sb[:], start=True, stop=True)
                    nc.vector.tensor_tensor(
                        out=tabA_sb[:, g0:g0 + 8, :]
                            .rearrange("p b c -> p (b c)"),
                        in0=pa[:], in1=biasA_sb[:, :8 * ch],
                        op=mybir.AluOpType.add)
                for g0 in range(0, nw, 8):
                    pb = psum.tile([128, 8 * ch], f32, tag="pt")
                    for jj in range(8):
                        w = g0 + jj
                        nc.tensor.matmul(pb[:, jj * ch:(jj + 1) * ch],
                                         hT2_sb[:, w * 128:(w + 1) * 128],
                                         Wa1d_sb[:], start=True, stop=True)
                    nc.scalar.copy(
                        tabB_sb[:, g0:g0 + 8, :]
                            .rearrange("p b c -> p (b c)"), pb[:])

            with tc.tile_pool(name="ohp", bufs=3) as ohp, \
                 tc.tile_pool(name="xp", bufs=8) as xp, \
                 tc.tile_pool(name="psum_e", bufs=8, space="PSUM") as psume:
                for t0 in range(0, T, CHUNK):
                    nt = min(CHUNK, T - t0)
                    oh_sb = ohp.tile([128, CHUNK, 2, 128], fp8, tag="oh")
                    ohf = oh_sb[:].rearrange("p t s q -> p (t s q)")
                    half = (nt // 2) * 256
                    nc.sync.dma_start(ohf[:, :half],
                                      oh[:, t0 * 256:t0 * 256 + half])
                    nc.sync.dma_start(ohf[:, half:nt * 256],
                                      oh[:, t0 * 256 + half:(t0 + nt) * 256])
                    for g in range(nt // GT):
                        ps = psume.tile([128, GT, ch], f32, tag="ps")
                        for q in range(GT):
                            kl = g * GT + q
                            k = t0 + kl
                            nc.tensor.matmul(ps[:, q, :],
                                             oh_sb[:, kl, 0, :],
                                             tabA_sb[:, int(a_k[k]), :],
                                             start=True, stop=False)
                            nc.tensor.matmul(ps[:, q, :],
                                             oh_sb[:, kl, 1, :],
                                             tabB_sb[:, int(b_k[k]), :],
                                             start=False, stop=True)
                        x = xp.tile([128, GT, ch], bf16, tag="x")
                        xf = x[:].rearrange("p b c -> p (b c)")
                        nc.scalar.activation(
                            out=xf, in_=ps[:].rearrange("p b c -> p (b c)"),
                            func=AF.Lrelu, alpha=0.01)
                        k0 = t0 + g * GT
                        nc.vector.tensor_reduce(
                            out=rp[:, k0:k0 + GT], in_=x[:, :, :p_pos],
                            axis=mybir.AxisListType.X, op=mybir.AluOpType.add)
                        nc.vector.tensor_reduce(
                            out=rn[:, k0:k0 + GT], in_=x[:, :, p_pos:],
                            axis=mybir.AxisListType.X, op=mybir.AluOpType.add)

                nc.vector.tensor_tensor(out=out_sb[:], in0=rp[:], in1=rn[:],
                                        op=mybir.AluOpType.subtract)
                nc.scalar.activation(out=out_sb[:], in_=out_sb[:],
                                     func=AF.Copy, bias=float(b_a2))
                nc.sync.dma_start(outp[:], out_sb[:])

    return nc


def full_cfg():
    return dict(n_feat=N_FEAT, ch=CH, n_node_pad=NODE_PAD)


def host_prep(cfg, node_feat, W_node, b_node, W_a1, b_a1, W_a2):
    """Shared (core-independent) inputs: weight folding + layout."""
    nf = cfg["n_feat"]
    ch = cfg["ch"]
    npad = cfg["n_node_pad"]

    w2 = np.asarray(W_a2, np.float32).reshape(-1)
    neg = w2 < 0
    perm = np.argsort(neg, kind="stable")  # positives (and zeros) first
    p_pos = int((~neg).sum())
    w2p = w2[perm]
    scale = np.abs(w2p).astype(np.float32)

    Wa1p = np.asarray(W_a1, np.float32)[:, perm]
    b1p = np.asarray(b_a1, np.float32)[perm]
    Wa1s = np.ascontiguousarray(Wa1p[:ch] * scale[None, :]).astype(BF16)
    Wa1d = np.ascontiguousarray(Wa1p[ch:] * scale[None, :]).astype(BF16)
    biasA = np.ascontiguousarray(
        np.tile((b1p * scale)[None, :], (128, 16))).astype(np.float32)

    n_nodes = node_feat.shape[0]
    nfT = np.zeros((nf + 1, npad), np.float32)
    nfT[:nf, :n_nodes] = np.asarray(node_feat, np.float32).T
    nfT[nf, :n_nodes] = 1.0
    nfT = nfT.astype(BF16)
    Wn = np.concatenate(
        [np.asarray(W_node, np.float32),
         np.asarray(b_node, np.float32)[None, :]], axis=0).astype(BF16)
    return dict(nfT=nfT, Wn=Wn, Wa1s=Wa1s, Wa1d=Wa1d, biasA=biasA), p_pos


def core_inputs(c, src, dst, a_k, b_k, T, slot_edge_c, nfT):
    """Per-core rotated node features + stacked one-hot input."""
    s_idx = np.nonzero(slot_edge_c >= 0)[0]
    e_idx = slot_edge_c[s_idx]
    tile_of = s_idx // 128
    q_of = s_idx % 128
    rowA = src[e_idx] - a_k[tile_of] * 128
    wd_phys = (b_k[tile_of] + BAND * c) % NW
    rowB = dst[e_idx] - wd_phys * 128
    assert (rowA >= 0).all() and (rowA < 128).all()
    assert (rowB >= 0).all() and (rowB < 128).all()
    ohAB = np.zeros((128, T, 2, 128), FP8)
    ohAB[rowA, tile_of, 0, q_of] = 1
    ohAB[rowB, tile_of, 1, q_of] = 1

    kdim = nfT.shape[0]
    nfw = nfT.reshape(kdim, NW, 128)
    nfT2 = np.ascontiguousarray(
        np.take(nfw, (np.arange(NW) + BAND * c) % NW, axis=1)
        .reshape(kdim, NW * 128))
    return {"ohAB": ohAB.reshape(128, T * 2 * 128), "nfT2": nfT2}


_PROG_CACHE = {}
LAST_RESULTS = None


def kernel(node_feat, edge_feat, src, dst, W_node, b_node, W_edge, b_edge,
           W_a1, b_a1, W_a2, b_a2, layer_num):
    global LAST_RESULTS
    assert int(layer_num) >= 1
    cfg = full_cfg()

    node_feat = np.asarray(node_feat)
    src = np.asarray(src).astype(np.int64)
    dst = np.asarray(dst).astype(np.int64)

    shared, p_pos = host_prep(cfg, node_feat, W_node, b_node, W_a1, b_a1,
                              W_a2)
    assert 0 < p_pos < CH
    b2 = float(np.asarray(b_a2, np.float32).reshape(-1)[0])
    a_k, b_k, T, slot_edge = plan_shards(src, dst)

    key = (p_pos, b2, T, hash(a_k.tobytes()), hash(b_k.tobytes()))
    nc = _PROG_CACHE.get(key)
    if nc is None:
        nc = build_program(cfg, p_pos, b2, a_k, b_k, T)
        nc.finalize()
        _PROG_CACHE[key] = nc

    in_maps = []
    for c in range(N_CORES):
        m = dict(shared)
        m.update(core_inputs(c, src, dst, a_k, b_k, T, slot_edge[c],
                             shared["nfT"]))
        in_maps.append(m)

    from concourse.bass_utils import run_bass_kernel_spmd
    trace = bool(os.environ.get("GAT_TRACE"))
    res = run_bass_kernel_spmd(nc, in_maps, core_ids=list(range(N_CORES)),
                               trace=trace)
    LAST_RESULTS = res

    e = np.zeros(N_EDGES, np.float32)
    for c in range(N_CORES):
        out = res.results[c]["out"]  # [128, T]
        se = slot_edge[c]
        s_idx = np.nonzero(se >= 0)[0]
        e[se[s_idx]] = out[s_idx % 128, s_idx // 128]
    return e.reshape(N_EDGES, 1)


# revision 12
# speedup vs baseline: 1.3568x; 1.3568x over previous
"""GATv2 edge-score kernel for 8 TRN2 NeuronCores (two-hop one-hot, no gather).

Math: the reference's layer loop is idempotent (h never changes) and eh is
unused, so the output is one pass:
    h   = node_feat @ W_node + b_node                       [N, C]
    e_j = leaky_relu(cat(h[src_j], h[dst_j]) @ W_a1 + b_a1) @ W_a2 + b_a2

Factored into per-node tables with |w2| folded in (leaky_relu is positively
homogeneous) and the node/attention weights pre-multiplied on host:
    tabA[n] = [node_feat[n], 1] @ WfA        (WfA = [W_node;b_node]@Wa1s'+b1')
    tabB[n] = [node_feat[n], 1] @ WfB
    e_j = sum_{c in pos} lrelu(u_jc) - sum_{c in neg} lrelu(u_jc) + b_a2
    u_j = tabA[src_j] + tabB[dst_j]          (channels permuted pos-first)

Both gathers are one-hot matmuls.  Nodes are padded to 80 windows of 128.
Edges are grouped by (src-window ws, dst-window wd) blocks; a 128-edge
tile's u is two PE matmuls accumulating in PSUM:
    psum[slot, c] = ohA.T @ A_window(ws)  +  ohB.T @ B_window(wd)
One-hots are fp8e4 (exact for 0/1, halves DMA); tables are bf16.

SPMD trick: edge -> core by diagonal band c = ((wd - ws) mod 80) // 10, so
every core's tile list k has the SAME table indices a_k = ws and
b_k = (ws + j) mod 80.  Core c's B-table is built from nfT2 = node features
with window blocks rotated by 10c, so SBUF index b holds physical window
(b + 10c) mod 80.  The rotation lives in input data; the program is shared.
"""

import os
import numpy as np
import ml_dtypes

BF16 = ml_dtypes.bfloat16
FP8 = ml_dtypes.float8_e4m3

# ---- problem constants (hardcoded; grader supplies exactly this shape) ----
N_NODES = 10000
N_FEAT = 118
CH = 128
N_EDGES = 640000
N_CORES = 8
NW = 80                      # node windows of 128
NODE_PAD = NW * 128          # 10240
BAND = NW // N_CORES         # 10 dst-window diagonals per core
GT = 16                      # tiles per PSUM group (4 banks)
CHUNK = 32                   # tiles per one-hot DMA chunk


def plan_shards(src, dst):
    """Diagonal-band core assignment with a shared tile template.

    Returns (a_k, b_k, T, slot_edge) where tile k on every core uses
    A-window a_k and B-table index b_k, and slot_edge[c] maps slot ->
    global edge id (-1 pad).
    """
    ws = (src // 128).astype(np.int64)
    wd = (dst // 128).astype(np.int64)
    d = (wd - ws) % NW
    core = d // BAND
    j = d % BAND
    m = ws * BAND + j                      # template slot, 0..799

    cnt = np.bincount(m * N_CORES + core,
                      minlength=NW * BAND * N_CORES).reshape(-1, N_CORES)
    mx = cnt.max(axis=1)
    Q = np.where(mx > 0, -(-mx // 128), 0).astype(np.int64)   # tiles per slot
    K = np.concatenate([[0], np.cumsum(Q)])                   # tile base
    T_raw = int(K[-1])
    T = -(-T_raw // GT) * GT

    a_k = np.zeros(T, np.int64)
    b_k = np.zeros(T, np.int64)
    mm = np.nonzero(Q > 0)[0]
    tile_m = np.repeat(mm, Q[mm])
    a_k[:T_raw] = tile_m // BAND
    b_k[:T_raw] = (tile_m // BAND + tile_m % BAND) % NW

    # per-core slot assignment: sort edges by (core, m), rank within group
    order = np.lexsort((m, core))
    mo, co = m[order], core[order]
    gid = co * (NW * BAND) + mo
    start = np.zeros(len(gid), bool)
    start[0] = True
    start[1:] = gid[1:] != gid[:-1]
    gstart = np.nonzero(start)[0]
    rank = np.arange(len(gid)) - np.repeat(gstart, np.diff(
        np.concatenate([gstart, [len(gid)]])))
    slot = K[mo] * 128 + rank

    slot_edge = np.full((N_CORES, T * 128), -1, np.int64)
    slot_edge[co, slot] = order
    return a_k, b_k, T, slot_edge


def build_program(cfg, L, M, b_a2, a_k, b_k, T):
    """One SPMD Bass program shared by all cores.

    Channels are host-permuted to [L pos | M mid | L neg] with mid all one
    sign (s_mid applied on host into the epilogue sign), so the signed
    channel reduce is z = x[:, :L] - x[:, L+M:] (bf16, 2x DVE) plus a small
    mid-range reduce:  e = sum(z) - sum(mid) + b2.
    """
    import concourse.mybir as mybir
    import concourse.tile as tile
    from concourse import bacc

    f32 = mybir.dt.float32
    bf16 = mybir.dt.bfloat16
    fp8 = mybir.dt.float8e4
    AF = mybir.ActivationFunctionType

    nf = cfg["n_feat"]
    ch = cfg["ch"]
    npad = cfg["n_node_pad"]
    nw = npad // 128
    kdim = nf + 1
    assert T % GT == 0

    nc = bacc.Bacc("TRN2", target_bir_lowering=False)
    nfT = nc.declare_dram_parameter("nfT", [kdim, npad], bf16, isOutput=False)
    nfT2 = nc.declare_dram_parameter("nfT2", [kdim, npad], bf16,
                                     isOutput=False)
    WfA = nc.declare_dram_parameter("WfA", [kdim, ch], bf16, isOutput=False)
    WfB = nc.declare_dram_parameter("WfB", [kdim, ch], bf16, isOutput=False)
    oh = nc.declare_dram_parameter("ohAB", [128, T * 2 * 128], fp8,
                                   isOutput=False)
    outp = nc.declare_dram_parameter("out", [128, T], f32, isOutput=True)

    NSPLIT = 8  # nfT DMA chunks (queue spread)

    with tile.TileContext(nc) as tc:
        with tc.tile_pool(name="persist", bufs=1) as pers:
            tabA_sb = pers.tile([128, nw, ch], bf16)
            tabB_sb = pers.tile([128, nw, ch], bf16)
            out_sb = pers.tile([128, T], f32)
            rp = pers.tile([128, T], f32)
            rn = pers.tile([128, T], f32)

            with tc.tile_pool(name="pre", bufs=1) as pre, \
                 tc.tile_pool(name="psum_pre", bufs=2, space="PSUM") as psum:
                nfT_sb = pre.tile([kdim, npad], bf16)
                nfT2_sb = pre.tile([kdim, npad], bf16)
                step = npad // NSPLIT
                for s0 in range(0, npad, step):
                    nc.sync.dma_start(nfT_sb[:, s0:s0 + step],
                                      nfT[:, s0:s0 + step])
                    nc.sync.dma_start(nfT2_sb[:, s0:s0 + step],
                                      nfT2[:, s0:s0 + step])
                WfA_sb = pre.tile([kdim, ch], bf16)
                nc.sync.dma_start(WfA_sb[:], WfA[:])
                WfB_sb = pre.tile([kdim, ch], bf16)
                nc.sync.dma_start(WfB_sb[:], WfB[:])

                # tables straight from node features (weights pre-folded);
                # psum->bf16 casts split across Scalar and Vector engines
                for g0 in range(0, nw, 8):
                    for srcT, wT, dstT in (
                            (nfT_sb, WfA_sb, tabA_sb),
                            (nfT2_sb, WfB_sb, tabB_sb)):
                        pt = psum.tile([128, 8 * ch], f32, tag="pt")
                        for jj in range(8):
                            w = g0 + jj
                            nc.tensor.matmul(pt[:, jj * ch:(jj + 1) * ch],
                                             srcT[:, w * 128:(w + 1) * 128],
                                             wT[:], start=True, stop=True)
                        df = dstT[:, g0:g0 + 8, :].rearrange(
                            "p b c -> p (b c)")
                        nc.scalar.copy(df[:, :4 * ch], pt[:, :4 * ch])
                        nc.vector.tensor_copy(df[:, 4 * ch:], pt[:, 4 * ch:])

            with tc.tile_pool(name="ohp", bufs=4) as ohp, \
                 tc.tile_pool(name="xp", bufs=4) as xp, \
                 tc.tile_pool(name="zp", bufs=4) as zp, \
                 tc.tile_pool(name="psum_e", bufs=2, space="PSUM") as psume:
                if M == 0:
                    nc.gpsimd.memset(rn[:], 0.0)
                for t0 in range(0, T, CHUNK):
                    nt = min(CHUNK, T - t0)
                    oh_sb = ohp.tile([128, CHUNK, 2, 128], fp8, tag="oh")
                    ohf = oh_sb[:].rearrange("p t s q -> p (t s q)")
                    for q0 in range(0, nt, 8):
                        qn = min(8, nt - q0)
                        nc.sync.dma_start(
                            ohf[:, q0 * 256:(q0 + qn) * 256],
                            oh[:, (t0 + q0) * 256:(t0 + q0 + qn) * 256])
                    for g in range(nt // GT):
                        ps = psume.tile([128, GT, ch], f32, tag="ps")
                        for q in range(GT):
                            kl = g * GT + q
                            k = t0 + kl
                            nc.tensor.matmul(ps[:, q, :],
                                             oh_sb[:, kl, 0, :],
                                             tabA_sb[:, int(a_k[k]), :],
                                             start=True, stop=False)
                            nc.tensor.matmul(ps[:, q, :],
                                             oh_sb[:, kl, 1, :],
                                             tabB_sb[:, int(b_k[k]), :],
                                             start=False, stop=True)
                        x = xp.tile([128, GT, ch], bf16, tag="x")
                        xf = x[:].rearrange("p b c -> p (b c)")
                        nc.scalar.activation(
                            out=xf, in_=ps[:].rearrange("p b c -> p (b c)"),
                            func=AF.Lrelu, alpha=0.01)
                        k0 = t0 + g * GT
                        z = zp.tile([128, GT, L], bf16, tag="z")
                        nc.vector.tensor_tensor(
                            out=z[:], in0=x[:, :, :L], in1=x[:, :, L + M:],
                            op=mybir.AluOpType.subtract)
                        nc.vector.tensor_reduce(
                            out=rp[:, k0:k0 + GT], in_=z[:],
                            axis=mybir.AxisListType.X, op=mybir.AluOpType.add)
                        if M > 0:
                            nc.vector.tensor_reduce(
                                out=rn[:, k0:k0 + GT], in_=x[:, :, L:L + M],
                                axis=mybir.AxisListType.X,
                                op=mybir.AluOpType.add)

                mid_op = (mybir.AluOpType.add if M > 0 and cfg["mid_pos"]
                          else mybir.AluOpType.subtract)
                nc.vector.tensor_tensor(out=out_sb[:], in0=rp[:], in1=rn[:],
                                        op=mid_op)
                nc.scalar.activation(out=out_sb[:], in_=out_sb[:],
                                     func=AF.Copy, bias=float(b_a2))
                nc.sync.dma_start(outp[:], out_sb[:])

    return nc


def full_cfg():
    return dict(n_feat=N_FEAT, ch=CH, n_node_pad=NODE_PAD)


def host_prep(cfg, node_feat, W_node, b_node, W_a1, b_a1, W_a2):
    """Shared (core-independent) inputs: weight folding + layout."""
    nf = cfg["n_feat"]
    ch = cfg["ch"]
    npad = cfg["n_node_pad"]

    w2 = np.asarray(W_a2, np.float32).reshape(-1)
    neg = w2 < 0
    pos_idx = np.nonzero(~neg)[0]
    neg_idx = np.nonzero(neg)[0]
    p_pos = len(pos_idx)
    L = min(p_pos, CH - p_pos)
    M = CH - 2 * L
    mid_pos = p_pos > CH - p_pos
    # channel layout [L pos | M mid (all one sign) | L neg]
    if mid_pos:
        perm = np.concatenate([pos_idx[:L], pos_idx[L:], neg_idx])
    else:
        perm = np.concatenate([pos_idx, neg_idx[:M], neg_idx[M:]])
    w2p = w2[perm]
    scale = np.abs(w2p).astype(np.float32)

    Wa1p = np.asarray(W_a1, np.float32)[:, perm]
    b1p = np.asarray(b_a1, np.float32)[perm]
    Wa1s = Wa1p[:ch] * scale[None, :]
    Wa1d = Wa1p[ch:] * scale[None, :]
    Wn = np.asarray(W_node, np.float32)
    bn = np.asarray(b_node, np.float32)

    # [node_feat,1] @ WfA == (node_feat@W_node+b_node) @ Wa1s + b1*scale
    WfA = np.concatenate([Wn @ Wa1s, (bn @ Wa1s + b1p * scale)[None, :]],
                         axis=0).astype(BF16)
    WfB = np.concatenate([Wn @ Wa1d, (bn @ Wa1d)[None, :]],
                         axis=0).astype(BF16)

    n_nodes = node_feat.shape[0]
    nfT = np.zeros((nf + 1, npad), np.float32)
    nfT[:nf, :n_nodes] = np.asarray(node_feat, np.float32).T
    nfT[nf, :n_nodes] = 1.0
    nfT = nfT.astype(BF16)
    return dict(nfT=nfT, WfA=WfA, WfB=WfB), (L, M, mid_pos)


def core_inputs(c, src, dst, a_k, b_k, T, slot_edge_c, nfT):
    """Per-core rotated node features + stacked one-hot input."""
    s_idx = np.nonzero(slot_edge_c >= 0)[0]
    e_idx = slot_edge_c[s_idx]
    tile_of = s_idx // 128
    q_of = s_idx % 128
    rowA = src[e_idx] - a_k[tile_of] * 128
    wd_phys = (b_k[tile_of] + BAND * c) % NW
    rowB = dst[e_idx] - wd_phys * 128
    assert (rowA >= 0).all() and (rowA < 128).all()
    assert (rowB >= 0).all() and (rowB < 128).all()
    ohAB = np.zeros((128, T, 2, 128), FP8)
    ohAB[rowA, tile_of, 0, q_of] = 1
    ohAB[rowB, tile_of, 1, q_of] = 1

    kdim = nfT.shape[0]
    nfw = nfT.reshape(kdim, NW, 128)
    nfT2 = np.ascontiguousarray(
        np.take(nfw, (np.arange(NW) + BAND * c) % NW, axis=1)
        .reshape(kdim, NW * 128))
    return {"ohAB": ohAB.reshape(128, T * 2 * 128), "nfT2": nfT2}


_PROG_CACHE = {}
LAST_RESULTS = None


def kernel(node_feat, edge_feat, src, dst, W_node, b_node, W_edge, b_edge,
           W_a1, b_a1, W_a2, b_a2, layer_num):
    global LAST_RESULTS
    assert int(layer_num) >= 1
    cfg = full_cfg()

    node_feat = np.asarray(node_feat)
    src = np.asarray(src).astype(np.int64)
    dst = np.asarray(dst).astype(np.int64)

    shared, (L, M, mid_pos) = host_prep(cfg, node_feat, W_node, b_node,
                                        W_a1, b_a1, W_a2)
    assert L > 0
    cfg["mid_pos"] = mid_pos
    b2 = float(np.asarray(b_a2, np.float32).reshape(-1)[0])
    a_k, b_k, T, slot_edge = plan_shards(src, dst)

    key = (L, M, mid_pos, b2, T, hash(a_k.tobytes()), hash(b_k.tobytes()))
    nc = _PROG_CACHE.get(key)
    if nc is None:
        nc = build_program(cfg, L, M, b2, a_k, b_k, T)
        nc.finalize()
        _PROG_CACHE[key] = nc

    in_maps = []
    for c in range(N_CORES):
        m = dict(shared)
        m.update(core_inputs(c, src, dst, a_k, b_k, T, slot_edge[c],
                             shared["nfT"]))
        in_maps.append(m)

    from concourse.bass_utils import run_bass_kernel_spmd
    trace = bool(os.environ.get("GAT_TRACE"))
    res = run_bass_kernel_spmd(nc, in_maps, core_ids=list(range(N_CORES)),
                               trace=trace)
    LAST_RESULTS = res

    e = np.zeros(N_EDGES, np.float32)
    for c in range(N_CORES):
        out = res.results[c]["out"]  # [128, T]
        se = slot_edge[c]
        s_idx = np.nonzero(se >= 0)[0]
        e[se[s_idx]] = out[s_idx % 128, s_idx // 128]
    return e.reshape(N_EDGES, 1)
